# revision 41
# baseline (speedup 1.0000x reference)
"""MoE layer (top-2 routing, 8 experts, capacity-limited) on 8 TRN2 NeuronCores.

Sharding strategy (expert-parallel, per the problem's sharding hint):
  - Routing (gate logits, softmax, top-2, capacity) is computed on host in
    fp32 with semantics bit-matching the reference (stable top-k, slot-major
    capacity ordering).
  - Token dispatch: for each expert e, the tokens routed to it are gathered,
    cast to bf16 and laid out transposed (d-major) on the host, then shipped
    to core e.  Core e holds W1[e]/W2[e] (bf16) resident in SBUF and runs the
    full FFN  w * (gelu(x @ W1 + b1) @ W2 + b2)  with fp32 PSUM accumulation.
  - Un-shard: the per-expert result rows are combined on host with two
    vectorized gathers (one per routing slot) and an add.

The device kernel is a static, fully-unrolled Tile program; one NEFF is
compiled and run SPMD on cores 0-7.
"""

import math

import numpy as np
import ml_dtypes

E = 8
K = 2
CAP_FACTOR = 1.25
TOK_BLK = 512  # moving-operand free dim per matmul (fp32 PSUM bank limit)

_NC_CACHE: dict = {}


def _block_plan(NP: int):
    """Token blocks in processing order: big blocks first (most PE work per
    DMA byte while weights stream), small remainder last (short tail)."""
    blocks = []
    j0 = 0
    while j0 < NP:
        tb = min(TOK_BLK, NP - j0)
        blocks.append((j0, tb))
        j0 += tb
    blocks.sort(key=lambda jt: -jt[1])
    return blocks


# ---------------------------------------------------------------------------
# Host routing (replicates the reference's discrete decisions exactly)
# ---------------------------------------------------------------------------

def _route(x_flat: np.ndarray, gate_w: np.ndarray):
    N, D = x_flat.shape
    logits = x_flat @ gate_w.T  # fp32 (N, E)
    m = logits.max(-1, keepdims=True)
    probs = np.exp(logits - m)
    probs /= probs.sum(-1, keepdims=True)

    # stable top-k: ties resolved to the lower index, like jax.lax.top_k
    top_i = np.argsort(-probs, axis=-1, kind="stable")[:, :K]  # (N, K)
    top_w = np.take_along_axis(probs, top_i, axis=-1)
    disp_w = top_w / (top_w.sum(-1, keepdims=True) + 1e-9)

    capacity = max(1, int(math.ceil(CAP_FACTOR * N * K / E)))
    idx_flat = top_i.T.reshape(-1)  # (K*N,) slot-major
    oh = idx_flat[:, None] == np.arange(E)[None, :]
    pos = (np.cumsum(oh, axis=0) * oh).sum(-1) - 1  # arrival position per expert
    keep_flat = pos < capacity  # (K*N,)

    # per-expert gather lists, in slot-major token order (kept only)
    tok_lists, w_lists = [], []
    flat_pos = np.zeros(K * N, dtype=np.int64)
    for e in range(E):
        sel = np.flatnonzero((idx_flat == e) & keep_flat)
        flat_pos[sel] = np.arange(sel.size)
        tok = sel % N
        slot = sel // N
        tok_lists.append(tok)
        w_lists.append(disp_w[tok, slot].astype(np.float32))

    # Switch-style aux loss
    assigned = oh.reshape(K, N, E).sum(0) > 0
    f = assigned.mean(0, dtype=np.float32)
    p_mean = probs.mean(0, dtype=np.float32)
    aux = np.float32(E * np.sum(f * p_mean, dtype=np.float32))

    return top_i, keep_flat, flat_pos, tok_lists, w_lists, aux


# ---------------------------------------------------------------------------
# Device kernel (built once per padded-size, shared SPMD across 8 cores)
# ---------------------------------------------------------------------------

def _build_nc(NP: int, D: int, DFF: int):
    import concourse.bass as bass  # noqa: F401
    import concourse.tile as tile
    from concourse import bacc, mybir
    from contextlib import ExitStack

    KT = D // 128     # k-tiles for the first matmul contraction
    FT = DFF // 128   # ff-tiles (contraction tiles for the second matmul)
    NT = NP // 128

    bf16 = mybir.dt.bfloat16
    f32 = mybir.dt.float32
    AF = mybir.ActivationFunctionType
    ALU = mybir.AluOpType

    nc = bacc.Bacc(
        "TRN2",
        target_bir_lowering=False,
        debug=False,
        enable_asserts=False,
        num_devices=E,
    )

    # xh is block-packed: per partition, each block's (kt-major) slab is one
    # contiguous 8KB run -> near-peak DMA efficiency, one DMA per block
    xh = nc.dram_tensor("xh", [128, KT * NP], bf16, kind="ExternalInput").ap()
    w1 = nc.dram_tensor("w1", [128, KT, DFF], bf16, kind="ExternalInput").ap()
    w2 = nc.dram_tensor("w2", [128, FT, D], bf16, kind="ExternalInput").ap()
    b1t = nc.dram_tensor("b1t", [128, FT], f32, kind="ExternalInput").ap()
    b2r = nc.dram_tensor("b2r", [128, D], f32, kind="ExternalInput").ap()
    wt = nc.dram_tensor("wt", [128, NT], f32, kind="ExternalInput").ap()
    y = nc.dram_tensor("y", [NP, D], f32, kind="ExternalOutput").ap()

    blocks = _block_plan(NP)

    with tile.TileContext(nc) as tc, ExitStack() as ctx:
        const = ctx.enter_context(tc.tile_pool(name="const", bufs=1))
        xpool = ctx.enter_context(tc.tile_pool(name="x", bufs=2))
        hpool = ctx.enter_context(tc.tile_pool(name="h", bufs=2))
        bpool = ctx.enter_context(tc.tile_pool(name="b2w", bufs=2))
        opool = ctx.enter_context(tc.tile_pool(name="o", bufs=3))
        # bufs is per tag; PSUM budget: ps1 3 + ps2 4 + warm-up 1 = 8 banks
        ps1 = ctx.enter_context(tc.tile_pool(name="ps1", bufs=3, space="PSUM"))
        ps2 = ctx.enter_context(tc.tile_pool(name="ps2", bufs=4, space="PSUM"))

        # PE warm-up: the first ~10us of execution are DMA-ramp-bound with the
        # PE idle, and the HAM clock gate then holds the first ~3.4us of real
        # matmuls at 1.2GHz.  Dummy matmuls on a memset scratch tile keep the
        # PE's activity window busy through the DMA wait so real matmuls start
        # at 2.4GHz.  80 N=128 matmuls span ~7us — ending just before the
        # measured ~17us first-data arrival, inside the ~3.4us re-throttle
        # window.
        wpool = ctx.enter_context(tc.tile_pool(name="warm", bufs=1))
        wpsp = ctx.enter_context(tc.tile_pool(name="warmps", bufs=1, space="PSUM"))
        warm_sb = wpool.tile([128, 128], bf16, name="warm_sb")
        nc.vector.memset(warm_sb, 0)
        warm_ps = wpsp.tile([128, 128], f32, name="warm_ps")
        for _ in range(80):
            nc.tensor.matmul(warm_ps, lhsT=warm_sb, rhs=warm_sb, start=True, stop=True)

        # Each dma_start costs ~0.6us of serialized issue time on its engine's
        # queue, so DMAs are kept few and spread across the three queues that
        # can issue them: W1/W2/y on gpsimd, x on sync, biases on scalar.
        #
        # W1 in (ff-group x kt-half) chunks, DMA'd ff-group-major so arrival
        # order matches phase-1 consumption order (psum ft needs only the
        # chunks of its own ff-group across all kt)
        FG = 4                  # ff-tiles per group
        NG = FT // FG           # number of ff groups
        KH = KT // 2            # kt per chunk
        w1_t = {}

        def load_w1_group(g):
            for kh in range(2):
                w1c = const.tile(
                    [128, KH * FG * 128], bf16, name=f"w1c{g}_{kh}", tag=f"w1c{g}_{kh}"
                )
                # single queue (gpsimd): issuing these from scalar delays the
                # gelus queued behind them on that engine (measured +12us)
                nc.gpsimd.dma_start(
                    w1c.rearrange("p (k f) -> p k f", k=KH),
                    w1[:, kh * KH : (kh + 1) * KH, g * FG * 128 : (g + 1) * FG * 128],
                )
                w1_t[(g, kh)] = w1c

        def w1_slice(ft, kt):
            g, fi = divmod(ft, FG)
            kh, ki = divmod(kt, KH)
            off = ki * FG * 128 + fi * 128
            return w1_t[(g, kh)][:, off : off + 128]

        # lazily-loaded small/late tensors (keep the critical W1+x DMA window
        # free of competing traffic)
        lazy: dict = {}

        def get_b1():
            if "b1" not in lazy:
                t = const.tile([128, FT], f32, name="b1_sb")
                nc.sync.dma_start(t, b1t)
                lazy["b1"] = t
            return lazy["b1"]

        def get_epi():
            if "b2" not in lazy:
                t = const.tile([128, D], f32, name="b2_sb")
                nc.sync.dma_start(t, b2r)
                lazy["b2"] = t
                t2 = const.tile([128, NT], f32, name="wt_sb")
                nc.sync.dma_start(t2, wt)
                lazy["wt"] = t2
            return lazy["b2"], lazy["wt"]

        FW = 4                  # ff-tiles per W2 chunk
        w2_t = []

        def load_w2():
            for c in range(FT // FW):
                w2c = const.tile([128, FW * D], bf16, name=f"w2c{c}", tag=f"w2c{c}")
                nc.gpsimd.dma_start(
                    w2c.rearrange("p (k f) -> p k f", k=FW),
                    w2[:, c * FW : (c + 1) * FW, :],
                )
                w2_t.append(w2c)

        def w2_slice(ft, lo, hi):
            c, fi = divmod(ft, FW)
            return w2_t[c][:, fi * D + lo : fi * D + hi]

        def emit_phase1(j0, tb, xoff, first=False):
            if first:
                # first block: x in kt-pair quarters interleaved with the W1
                # group loads, so the first matmuls only wait for ~500KB
                xq = []
                for q in range(KT // 2):
                    xc = xpool.tile([128, 2 * tb], bf16, name=f"xq{q}", tag=f"xq{q}")
                    nc.sync.dma_start(
                        xc, xh[:, xoff + q * 2 * tb : xoff + (q + 1) * 2 * tb]
                    )
                    if q < NG:
                        load_w1_group(q)
                    xq.append(xc)
                for g in range(KT // 2, NG):
                    load_w1_group(g)

                def rhs_slice(kt):
                    return xq[kt // 2][:, (kt % 2) * tb : (kt % 2 + 1) * tb]
            else:
                xt = xpool.tile([128, KT * tb], bf16, name="xt", tag="xt")
                nc.sync.dma_start(xt, xh[:, xoff : xoff + KT * tb])

                def rhs_slice(kt):
                    return xt[:, kt * tb : (kt + 1) * tb]

            b1_sb = get_b1()
            # h = gelu(x @ W1 + b1), laid out (ff, tok)
            h_tiles = []
            for ft in range(FT):
                ps = ps1.tile([128, tb], f32, name="ps", tag="ps1")
                for kt in range(KT):
                    nc.tensor.matmul(
                        ps,
                        lhsT=w1_slice(ft, kt),
                        rhs=rhs_slice(kt),
                        start=(kt == 0),
                        stop=(kt == KT - 1),
                    )
                ht = hpool.tile([128, tb], bf16, name="ht", tag=f"h{ft}")
                nc.scalar.activation(ht, ps, AF.Gelu, bias=b1_sb[:, ft : ft + 1])
                h_tiles.append(ht)
                if first and ft in (FG * 2 - 1, FG * 3 - 1):
                    # bridge the W1-arrival waits so the HAM activity window
                    # stays busy and the clock gate never re-throttles
                    for _ in range(12):
                        nc.tensor.matmul(
                            warm_ps, lhsT=warm_sb, rhs=warm_sb,
                            start=True, stop=True,
                        )
            return h_tiles

        def emit_phase2(j0, tb, h_tiles, last=False, penult=False):
            if not w2_t:
                load_w2()
            b2_sb, wt_sb = get_epi()
            # y = w * (h.T @ W2 + b2), laid out (tok, d); per-sub-group output
            # tiles + DMAs (on the mostly-idle gpsimd queue) so results drain
            # as soon as their two epilogue ops finish
            nsg = tb // 128
            for sg in range(nsg):
                g = j0 // 128 + sg
                wcol = wt_sb[:, g : g + 1]
                b2w = bpool.tile([128, D], f32, name="b2w", tag="b2w")
                nc.vector.tensor_scalar_mul(b2w, b2_sb, wcol)
                # final sub-group: separate half tiles so each half's output
                # DMA starts right after its own epilogue op, shortening the
                # end-of-kernel drain wait
                split = last and sg == nsg - 1
                if not split:
                    outt = opool.tile([128, D], f32, name="outt", tag="out")
                for half in range(D // 512):
                    p2 = ps2.tile([128, 512], f32, name="p2", tag="ps2")
                    for ft in range(FT):
                        nc.tensor.matmul(
                            p2,
                            lhsT=h_tiles[ft][:, sg * 128 : (sg + 1) * 128],
                            rhs=w2_slice(ft, half * 512, half * 512 + 512),
                            start=(ft == 0),
                            stop=(ft == FT - 1),
                        )
                    if split:
                        outh = opool.tile([128, 512], f32, name="outh", tag="outh")
                        nc.vector.scalar_tensor_tensor(
                            outh, p2, wcol, b2w[:, half * 512 : half * 512 + 512],
                            op0=ALU.mult, op1=ALU.add,
                        )
                        eng = nc.gpsimd if half == 0 else nc.sync
                        eng.dma_start(
                            y[j0 + sg * 128 : j0 + (sg + 1) * 128,
                              half * 512 : half * 512 + 512],
                            outh,
                        )
                    else:
                        nc.vector.scalar_tensor_tensor(
                            outt[:, half * 512 : half * 512 + 512],
                            p2,
                            wcol,
                            b2w[:, half * 512 : half * 512 + 512],
                            op0=ALU.mult,
                            op1=ALU.add,
                        )
                if not split:
                    # alternate output queues by sub-group parity so the
                    # end-of-kernel flush drains through both queues; the
                    # penultimate block's last sg goes via scalar (idle after
                    # the final gelu) for a 3-way tail flush
                    if penult and sg == nsg - 1:
                        eng = nc.scalar
                    else:
                        eng = nc.gpsimd if sg % 2 == 0 else nc.sync
                    eng.dma_start(
                        y[j0 + sg * 128 : j0 + (sg + 1) * 128, :], outt
                    )

        # software pipeline: phase 2 runs one block behind phase 1, so the
        # PE's early work only needs W1 + x while W2 is still streaming in
        pending = None
        xoff = 0
        for bi, (j0, tb) in enumerate(blocks):
            h_tiles = emit_phase1(j0, tb, xoff, first=(bi == 0))
            xoff += KT * tb
            if pending is not None:
                emit_phase2(*pending, penult=(bi == len(blocks) - 1))
            pending = (j0, tb, h_tiles)
        emit_phase2(*pending, last=True)

    nc.compile()
    return nc


# ---------------------------------------------------------------------------
# Entry point
# ---------------------------------------------------------------------------

_last_run = None  # BassKernelResults of the most recent device launch


def _ensure_axon_hooks():
    # bass_utils imports antenv.axon_hooks unconditionally when BASS_TRACE is
    # set under axon; provide a no-op registry if the image lacks the module
    try:
        import antenv.axon_hooks  # noqa: F401
    except Exception:
        import sys as _sys
        import types as _types

        m = _types.ModuleType("antenv.axon_hooks")
        m._h = None
        m.set_axon_ntff_profile_hook = lambda h: setattr(m, "_h", h)
        m.get_axon_ntff_profile_hook = lambda: getattr(m, "_h", None)
        _sys.modules["antenv.axon_hooks"] = m


def kernel(x, gate_w, W1, b1, W2, b2):
    _ensure_axon_hooks()
    from concourse.bass_utils import run_bass_kernel_spmd

    x = np.asarray(x, dtype=np.float32)
    gate_w = np.asarray(gate_w, dtype=np.float32)
    W1 = np.asarray(W1, dtype=np.float32)
    b1 = np.asarray(b1, dtype=np.float32)
    W2 = np.asarray(W2, dtype=np.float32)
    b2 = np.asarray(b2, dtype=np.float32)

    B, T, D = x.shape
    DFF = W1.shape[2]
    N = B * T
    x_flat = x.reshape(N, D)

    top_i, keep_flat, flat_pos, tok_lists, w_lists, aux = _route(x_flat, gate_w)

    NP = max(128, max(-(-len(t) // 128) * 128 for t in tok_lists))
    KT, FT, NT = D // 128, DFF // 128, NP // 128

    key = (NP, D, DFF)
    if key not in _NC_CACHE:
        _NC_CACHE[key] = _build_nc(NP, D, DFF)
    nc = _NC_CACHE[key]

    bf16 = ml_dtypes.bfloat16
    x_bf = x_flat.astype(bf16)
    blocks = _block_plan(NP)

    in_maps = []
    for e in range(E):
        tok = tok_lists[e]
        n = len(tok)
        xg = np.zeros((NP, D), dtype=bf16)
        xg[:n] = x_bf[tok]
        # (NP, D) -> (128, KT, NP): xT[p, kt, j] = xg[j, kt*128 + p]
        xT_h = xg.reshape(NP, KT, 128).transpose(2, 1, 0)
        # block-packed: per partition, block (j0, tb) occupies one contiguous
        # kt-major slab [kt*tb + j]
        xh_h = np.empty((128, KT * NP), dtype=bf16)
        off = 0
        for j0, tb in blocks:
            xh_h[:, off : off + KT * tb] = xT_h[:, :, j0 : j0 + tb].reshape(128, KT * tb)
            off += KT * tb
        w1_h = np.ascontiguousarray(
            W1[e].reshape(KT, 128, DFF).transpose(1, 0, 2).astype(bf16)
        )
        w2_h = np.ascontiguousarray(
            W2[e].reshape(FT, 128, D).transpose(1, 0, 2).astype(bf16)
        )
        b1_h = np.ascontiguousarray(b1[e].reshape(FT, 128).T)
        b2_h = np.ascontiguousarray(np.broadcast_to(b2[e], (128, D)))
        wt_h = np.zeros((NT, 128), dtype=np.float32)
        wt_h.reshape(-1)[:n] = w_lists[e]
        wt_h = np.ascontiguousarray(wt_h.T)
        in_maps.append(
            {"xh": xh_h, "w1": w1_h, "w2": w2_h, "b1t": b1_h, "b2r": b2_h, "wt": wt_h}
        )

    res = run_bass_kernel_spmd(nc, in_maps, core_ids=list(range(E)))
    global _last_run
    _last_run = res

    y_all = np.stack([r["y"] for r in res.results]).reshape(E * NP, D)

    out = np.zeros((N, D), dtype=np.float32)
    ar = np.arange(N)
    for k in range(K):
        kk = keep_flat[k * N : (k + 1) * N]
        src = top_i[:, k].astype(np.int64) * NP + flat_pos[k * N + ar]
        contrib = y_all[src]
        contrib[~kk] = 0.0
        out += contrib

    return out.reshape(B, T, D), aux


# revision 43
# speedup vs baseline: 1.0001x; 1.0001x over previous
"""MoE layer (top-2 routing, 8 experts, capacity-limited) on 8 TRN2 NeuronCores.

Sharding strategy (expert-parallel, per the problem's sharding hint):
  - Routing (gate logits, softmax, top-2, capacity) is computed on host in
    fp32 with semantics bit-matching the reference (stable top-k, slot-major
    capacity ordering).
  - Token dispatch: for each expert e, the tokens routed to it are gathered,
    cast to bf16 and laid out transposed (d-major) on the host, then shipped
    to core e.  Core e holds W1[e]/W2[e] (bf16) resident in SBUF and runs the
    full FFN  w * (gelu(x @ W1 + b1) @ W2 + b2)  with fp32 PSUM accumulation.
  - Un-shard: the per-expert result rows are combined on host with two
    vectorized gathers (one per routing slot) and an add.

The device kernel is a static, fully-unrolled Tile program; one NEFF is
compiled and run SPMD on cores 0-7.
"""

import math

import numpy as np
import ml_dtypes

E = 8
K = 2
CAP_FACTOR = 1.25
TOK_BLK = 512  # moving-operand free dim per matmul (fp32 PSUM bank limit)

_NC_CACHE: dict = {}


def _block_plan(NP: int):
    """Token blocks in processing order: big blocks first (most PE work per
    DMA byte while weights stream), small remainder last (short tail)."""
    blocks = []
    j0 = 0
    while j0 < NP:
        tb = min(TOK_BLK, NP - j0)
        blocks.append((j0, tb))
        j0 += tb
    blocks.sort(key=lambda jt: -jt[1])
    return blocks


# ---------------------------------------------------------------------------
# Host routing (replicates the reference's discrete decisions exactly)
# ---------------------------------------------------------------------------

def _route(x_flat: np.ndarray, gate_w: np.ndarray):
    N, D = x_flat.shape
    logits = x_flat @ gate_w.T  # fp32 (N, E)
    m = logits.max(-1, keepdims=True)
    probs = np.exp(logits - m)
    probs /= probs.sum(-1, keepdims=True)

    # stable top-k: ties resolved to the lower index, like jax.lax.top_k
    top_i = np.argsort(-probs, axis=-1, kind="stable")[:, :K]  # (N, K)
    top_w = np.take_along_axis(probs, top_i, axis=-1)
    disp_w = top_w / (top_w.sum(-1, keepdims=True) + 1e-9)

    capacity = max(1, int(math.ceil(CAP_FACTOR * N * K / E)))
    idx_flat = top_i.T.reshape(-1)  # (K*N,) slot-major
    oh = idx_flat[:, None] == np.arange(E)[None, :]
    pos = (np.cumsum(oh, axis=0) * oh).sum(-1) - 1  # arrival position per expert
    keep_flat = pos < capacity  # (K*N,)

    # per-expert gather lists, in slot-major token order (kept only)
    tok_lists, w_lists = [], []
    flat_pos = np.zeros(K * N, dtype=np.int64)
    for e in range(E):
        sel = np.flatnonzero((idx_flat == e) & keep_flat)
        flat_pos[sel] = np.arange(sel.size)
        tok = sel % N
        slot = sel // N
        tok_lists.append(tok)
        w_lists.append(disp_w[tok, slot].astype(np.float32))

    # Switch-style aux loss
    assigned = oh.reshape(K, N, E).sum(0) > 0
    f = assigned.mean(0, dtype=np.float32)
    p_mean = probs.mean(0, dtype=np.float32)
    aux = np.float32(E * np.sum(f * p_mean, dtype=np.float32))

    return top_i, keep_flat, flat_pos, tok_lists, w_lists, aux


# ---------------------------------------------------------------------------
# Device kernel (built once per padded-size, shared SPMD across 8 cores)
# ---------------------------------------------------------------------------

def _build_nc(NP: int, D: int, DFF: int):
    import concourse.bass as bass  # noqa: F401
    import concourse.tile as tile
    from concourse import bacc, mybir
    from contextlib import ExitStack

    KT = D // 128     # k-tiles for the first matmul contraction
    FT = DFF // 128   # ff-tiles (contraction tiles for the second matmul)
    NT = NP // 128

    bf16 = mybir.dt.bfloat16
    f32 = mybir.dt.float32
    AF = mybir.ActivationFunctionType
    ALU = mybir.AluOpType

    nc = bacc.Bacc(
        "TRN2",
        target_bir_lowering=False,
        debug=False,
        enable_asserts=False,
        num_devices=E,
    )

    # xh is block-packed: per partition, each block's (kt-major) slab is one
    # contiguous 8KB run -> near-peak DMA efficiency, one DMA per block
    xh = nc.dram_tensor("xh", [128, KT * NP], bf16, kind="ExternalInput").ap()
    w1 = nc.dram_tensor("w1", [128, KT, DFF], bf16, kind="ExternalInput").ap()
    w2 = nc.dram_tensor("w2", [128, FT, D], bf16, kind="ExternalInput").ap()
    b1t = nc.dram_tensor("b1t", [128, FT], f32, kind="ExternalInput").ap()
    b2r = nc.dram_tensor("b2r", [128, D], f32, kind="ExternalInput").ap()
    wt = nc.dram_tensor("wt", [128, NT], f32, kind="ExternalInput").ap()
    y = nc.dram_tensor("y", [NP, D], f32, kind="ExternalOutput").ap()

    blocks = _block_plan(NP)

    with tile.TileContext(nc) as tc, ExitStack() as ctx:
        const = ctx.enter_context(tc.tile_pool(name="const", bufs=1))
        xpool = ctx.enter_context(tc.tile_pool(name="x", bufs=2))
        hpool = ctx.enter_context(tc.tile_pool(name="h", bufs=2))
        bpool = ctx.enter_context(tc.tile_pool(name="b2w", bufs=2))
        opool = ctx.enter_context(tc.tile_pool(name="o", bufs=3))
        # bufs is per tag; PSUM budget: ps1 3 + ps2 4 + warm-up 1 = 8 banks
        ps1 = ctx.enter_context(tc.tile_pool(name="ps1", bufs=3, space="PSUM"))
        ps2 = ctx.enter_context(tc.tile_pool(name="ps2", bufs=4, space="PSUM"))

        # PE warm-up: the first ~10us of execution are DMA-ramp-bound with the
        # PE idle, and the HAM clock gate then holds the first ~3.4us of real
        # matmuls at 1.2GHz.  Dummy matmuls on a memset scratch tile keep the
        # PE's activity window busy through the DMA wait so real matmuls start
        # at 2.4GHz.  80 N=128 matmuls span ~7us — ending just before the
        # measured ~17us first-data arrival, inside the ~3.4us re-throttle
        # window.
        wpool = ctx.enter_context(tc.tile_pool(name="warm", bufs=1))
        wpsp = ctx.enter_context(tc.tile_pool(name="warmps", bufs=1, space="PSUM"))
        warm_sb = wpool.tile([128, 512], bf16, name="warm_sb")
        nc.vector.memset(warm_sb, 0)
        warm_ps = wpsp.tile([128, 512], f32, name="warm_ps")
        for _ in range(60):
            nc.tensor.matmul(
                warm_ps[:, :128], lhsT=warm_sb[:, :128], rhs=warm_sb[:, :128],
                start=True, stop=True,
            )
        for _ in range(10):
            # longer dummies stretch coverage to the first data-dependent
            # matmul so the HAM clock gate never re-throttles
            nc.tensor.matmul(
                warm_ps, lhsT=warm_sb[:, :128], rhs=warm_sb,
                start=True, stop=True,
            )

        # Each dma_start costs ~0.6us of serialized issue time on its engine's
        # queue, so DMAs are kept few and spread across the three queues that
        # can issue them: W1/W2/y on gpsimd, x on sync, biases on scalar.
        #
        # W1 in (ff-group x kt-half) chunks, DMA'd ff-group-major so arrival
        # order matches phase-1 consumption order (psum ft needs only the
        # chunks of its own ff-group across all kt)
        FG = 4                  # ff-tiles per group
        NG = FT // FG           # number of ff groups
        KH = KT // 2            # kt per chunk
        w1_t = {}

        def load_w1_group(g):
            for kh in range(2):
                w1c = const.tile(
                    [128, KH * FG * 128], bf16, name=f"w1c{g}_{kh}", tag=f"w1c{g}_{kh}"
                )
                # single queue (gpsimd): issuing these from scalar delays the
                # gelus queued behind them on that engine (measured +12us)
                nc.gpsimd.dma_start(
                    w1c.rearrange("p (k f) -> p k f", k=KH),
                    w1[:, kh * KH : (kh + 1) * KH, g * FG * 128 : (g + 1) * FG * 128],
                )
                w1_t[(g, kh)] = w1c

        def w1_slice(ft, kt):
            g, fi = divmod(ft, FG)
            kh, ki = divmod(kt, KH)
            off = ki * FG * 128 + fi * 128
            return w1_t[(g, kh)][:, off : off + 128]

        # lazily-loaded small/late tensors (keep the critical W1+x DMA window
        # free of competing traffic)
        lazy: dict = {}

        def get_b1():
            if "b1" not in lazy:
                t = const.tile([128, FT], f32, name="b1_sb")
                nc.sync.dma_start(t, b1t)
                lazy["b1"] = t
            return lazy["b1"]

        def get_epi():
            if "b2" not in lazy:
                t = const.tile([128, D], f32, name="b2_sb")
                nc.sync.dma_start(t, b2r)
                lazy["b2"] = t
                t2 = const.tile([128, NT], f32, name="wt_sb")
                nc.sync.dma_start(t2, wt)
                lazy["wt"] = t2
            return lazy["b2"], lazy["wt"]

        FW = 4                  # ff-tiles per W2 chunk
        w2_t = []

        def load_w2():
            for c in range(FT // FW):
                w2c = const.tile([128, FW * D], bf16, name=f"w2c{c}", tag=f"w2c{c}")
                nc.gpsimd.dma_start(
                    w2c.rearrange("p (k f) -> p k f", k=FW),
                    w2[:, c * FW : (c + 1) * FW, :],
                )
                w2_t.append(w2c)

        def w2_slice(ft, lo, hi):
            c, fi = divmod(ft, FW)
            return w2_t[c][:, fi * D + lo : fi * D + hi]

        def emit_phase1(j0, tb, xoff, first=False):
            if first:
                # first block: x in kt-pair quarters interleaved with the W1
                # group loads, so the first matmuls only wait for ~500KB
                xq = []
                for q in range(KT // 2):
                    xc = xpool.tile([128, 2 * tb], bf16, name=f"xq{q}", tag=f"xq{q}")
                    nc.sync.dma_start(
                        xc, xh[:, xoff + q * 2 * tb : xoff + (q + 1) * 2 * tb]
                    )
                    if q < NG:
                        load_w1_group(q)
                    xq.append(xc)
                for g in range(KT // 2, NG):
                    load_w1_group(g)

                def rhs_slice(kt):
                    return xq[kt // 2][:, (kt % 2) * tb : (kt % 2 + 1) * tb]
            else:
                xt = xpool.tile([128, KT * tb], bf16, name="xt", tag="xt")
                nc.sync.dma_start(xt, xh[:, xoff : xoff + KT * tb])

                def rhs_slice(kt):
                    return xt[:, kt * tb : (kt + 1) * tb]

            b1_sb = get_b1()
            # h = gelu(x @ W1 + b1), laid out (ff, tok)
            h_tiles = []
            for ft in range(FT):
                ps = ps1.tile([128, tb], f32, name="ps", tag="ps1")
                for kt in range(KT):
                    nc.tensor.matmul(
                        ps,
                        lhsT=w1_slice(ft, kt),
                        rhs=rhs_slice(kt),
                        start=(kt == 0),
                        stop=(kt == KT - 1),
                    )
                ht = hpool.tile([128, tb], bf16, name="ht", tag=f"h{ft}")
                nc.scalar.activation(ht, ps, AF.Gelu, bias=b1_sb[:, ft : ft + 1])
                h_tiles.append(ht)
            return h_tiles

        def emit_phase2(j0, tb, h_tiles, last=False, penult=False):
            if not w2_t:
                load_w2()
            b2_sb, wt_sb = get_epi()
            # y = w * (h.T @ W2 + b2), laid out (tok, d); per-sub-group output
            # tiles + DMAs (on the mostly-idle gpsimd queue) so results drain
            # as soon as their two epilogue ops finish
            nsg = tb // 128
            for sg in range(nsg):
                g = j0 // 128 + sg
                wcol = wt_sb[:, g : g + 1]
                b2w = bpool.tile([128, D], f32, name="b2w", tag="b2w")
                nc.vector.tensor_scalar_mul(b2w, b2_sb, wcol)
                # final sub-group: separate half tiles so each half's output
                # DMA starts right after its own epilogue op, shortening the
                # end-of-kernel drain wait
                split = last and sg == nsg - 1
                if not split:
                    outt = opool.tile([128, D], f32, name="outt", tag="out")
                for half in range(D // 512):
                    p2 = ps2.tile([128, 512], f32, name="p2", tag="ps2")
                    for ft in range(FT):
                        nc.tensor.matmul(
                            p2,
                            lhsT=h_tiles[ft][:, sg * 128 : (sg + 1) * 128],
                            rhs=w2_slice(ft, half * 512, half * 512 + 512),
                            start=(ft == 0),
                            stop=(ft == FT - 1),
                        )
                    if split:
                        outh = opool.tile([128, 512], f32, name="outh", tag="outh")
                        nc.vector.scalar_tensor_tensor(
                            outh, p2, wcol, b2w[:, half * 512 : half * 512 + 512],
                            op0=ALU.mult, op1=ALU.add,
                        )
                        eng = nc.gpsimd if half == 0 else nc.sync
                        eng.dma_start(
                            y[j0 + sg * 128 : j0 + (sg + 1) * 128,
                              half * 512 : half * 512 + 512],
                            outh,
                        )
                    else:
                        nc.vector.scalar_tensor_tensor(
                            outt[:, half * 512 : half * 512 + 512],
                            p2,
                            wcol,
                            b2w[:, half * 512 : half * 512 + 512],
                            op0=ALU.mult,
                            op1=ALU.add,
                        )
                if not split:
                    # alternate output queues by sub-group parity so the
                    # end-of-kernel flush drains through both queues; the
                    # penultimate block's last sg goes via scalar (idle after
                    # the final gelu) for a 3-way tail flush
                    if penult and sg == nsg - 1:
                        eng = nc.scalar
                    else:
                        eng = nc.gpsimd if sg % 2 == 0 else nc.sync
                    eng.dma_start(
                        y[j0 + sg * 128 : j0 + (sg + 1) * 128, :], outt
                    )

        # software pipeline: phase 2 runs one block behind phase 1, so the
        # PE's early work only needs W1 + x while W2 is still streaming in
        pending = None
        xoff = 0
        for bi, (j0, tb) in enumerate(blocks):
            h_tiles = emit_phase1(j0, tb, xoff, first=(bi == 0))
            xoff += KT * tb
            if pending is not None:
                emit_phase2(*pending, penult=(bi == len(blocks) - 1))
            pending = (j0, tb, h_tiles)
        emit_phase2(*pending, last=True)

    nc.compile()
    return nc


# ---------------------------------------------------------------------------
# Entry point
# ---------------------------------------------------------------------------

_last_run = None  # BassKernelResults of the most recent device launch


def _ensure_axon_hooks():
    # bass_utils imports antenv.axon_hooks unconditionally when BASS_TRACE is
    # set under axon; provide a no-op registry if the image lacks the module
    try:
        import antenv.axon_hooks  # noqa: F401
    except Exception:
        import sys as _sys
        import types as _types

        m = _types.ModuleType("antenv.axon_hooks")
        m._h = None
        m.set_axon_ntff_profile_hook = lambda h: setattr(m, "_h", h)
        m.get_axon_ntff_profile_hook = lambda: getattr(m, "_h", None)
        _sys.modules["antenv.axon_hooks"] = m


def kernel(x, gate_w, W1, b1, W2, b2):
    _ensure_axon_hooks()
    from concourse.bass_utils import run_bass_kernel_spmd

    x = np.asarray(x, dtype=np.float32)
    gate_w = np.asarray(gate_w, dtype=np.float32)
    W1 = np.asarray(W1, dtype=np.float32)
    b1 = np.asarray(b1, dtype=np.float32)
    W2 = np.asarray(W2, dtype=np.float32)
    b2 = np.asarray(b2, dtype=np.float32)

    B, T, D = x.shape
    DFF = W1.shape[2]
    N = B * T
    x_flat = x.reshape(N, D)

    top_i, keep_flat, flat_pos, tok_lists, w_lists, aux = _route(x_flat, gate_w)

    NP = max(128, max(-(-len(t) // 128) * 128 for t in tok_lists))
    KT, FT, NT = D // 128, DFF // 128, NP // 128

    key = (NP, D, DFF)
    if key not in _NC_CACHE:
        _NC_CACHE[key] = _build_nc(NP, D, DFF)
    nc = _NC_CACHE[key]

    bf16 = ml_dtypes.bfloat16
    x_bf = x_flat.astype(bf16)
    blocks = _block_plan(NP)

    in_maps = []
    for e in range(E):
        tok = tok_lists[e]
        n = len(tok)
        xg = np.zeros((NP, D), dtype=bf16)
        xg[:n] = x_bf[tok]
        # (NP, D) -> (128, KT, NP): xT[p, kt, j] = xg[j, kt*128 + p]
        xT_h = xg.reshape(NP, KT, 128).transpose(2, 1, 0)
        # block-packed: per partition, block (j0, tb) occupies one contiguous
        # kt-major slab [kt*tb + j]
        xh_h = np.empty((128, KT * NP), dtype=bf16)
        off = 0
        for j0, tb in blocks:
            xh_h[:, off : off + KT * tb] = xT_h[:, :, j0 : j0 + tb].reshape(128, KT * tb)
            off += KT * tb
        w1_h = np.ascontiguousarray(
            W1[e].reshape(KT, 128, DFF).transpose(1, 0, 2).astype(bf16)
        )
        w2_h = np.ascontiguousarray(
            W2[e].reshape(FT, 128, D).transpose(1, 0, 2).astype(bf16)
        )
        b1_h = np.ascontiguousarray(b1[e].reshape(FT, 128).T)
        b2_h = np.ascontiguousarray(np.broadcast_to(b2[e], (128, D)))
        wt_h = np.zeros((NT, 128), dtype=np.float32)
        wt_h.reshape(-1)[:n] = w_lists[e]
        wt_h = np.ascontiguousarray(wt_h.T)
        in_maps.append(
            {"xh": xh_h, "w1": w1_h, "w2": w2_h, "b1t": b1_h, "b2r": b2_h, "wt": wt_h}
        )

    res = run_bass_kernel_spmd(nc, in_maps, core_ids=list(range(E)))
    global _last_run
    _last_run = res

    y_all = np.stack([r["y"] for r in res.results]).reshape(E * NP, D)

    out = np.zeros((N, D), dtype=np.float32)
    ar = np.arange(N)
    for k in range(K):
        kk = keep_flat[k * N : (k + 1) * N]
        src = top_i[:, k].astype(np.int64) * NP + flat_pos[k * N + ar]
        contrib = y_all[src]
        contrib[~kk] = 0.0
        out += contrib

    return out.reshape(B, T, D), aux


# revision 44
# speedup vs baseline: 1.0043x; 1.0042x over previous
"""MoE layer (top-2 routing, 8 experts, capacity-limited) on 8 TRN2 NeuronCores.

Sharding strategy (expert-parallel, per the problem's sharding hint):
  - Routing (gate logits, softmax, top-2, capacity) is computed on host in
    fp32 with semantics bit-matching the reference (stable top-k, slot-major
    capacity ordering).
  - Token dispatch: for each expert e, the tokens routed to it are gathered,
    cast to bf16 and laid out transposed (d-major) on the host, then shipped
    to core e.  Core e holds W1[e]/W2[e] (bf16) resident in SBUF and runs the
    full FFN  w * (gelu(x @ W1 + b1) @ W2 + b2)  with fp32 PSUM accumulation.
  - Un-shard: the per-expert result rows are combined on host with two
    vectorized gathers (one per routing slot) and an add.

The device kernel is a static, fully-unrolled Tile program; one NEFF is
compiled and run SPMD on cores 0-7.
"""

import math

import numpy as np
import ml_dtypes

E = 8
K = 2
CAP_FACTOR = 1.25
TOK_BLK = 512  # moving-operand free dim per matmul (fp32 PSUM bank limit)

_NC_CACHE: dict = {}


def _block_plan(NP: int):
    """Token blocks in processing order: big blocks first (most PE work per
    DMA byte while weights stream), small remainder last (short tail)."""
    blocks = []
    j0 = 0
    while j0 < NP:
        tb = min(TOK_BLK, NP - j0)
        blocks.append((j0, tb))
        j0 += tb
    blocks.sort(key=lambda jt: -jt[1])
    return blocks


# ---------------------------------------------------------------------------
# Host routing (replicates the reference's discrete decisions exactly)
# ---------------------------------------------------------------------------

def _route(x_flat: np.ndarray, gate_w: np.ndarray):
    N, D = x_flat.shape
    logits = x_flat @ gate_w.T  # fp32 (N, E)
    m = logits.max(-1, keepdims=True)
    probs = np.exp(logits - m)
    probs /= probs.sum(-1, keepdims=True)

    # stable top-k: ties resolved to the lower index, like jax.lax.top_k
    top_i = np.argsort(-probs, axis=-1, kind="stable")[:, :K]  # (N, K)
    top_w = np.take_along_axis(probs, top_i, axis=-1)
    disp_w = top_w / (top_w.sum(-1, keepdims=True) + 1e-9)

    capacity = max(1, int(math.ceil(CAP_FACTOR * N * K / E)))
    idx_flat = top_i.T.reshape(-1)  # (K*N,) slot-major
    oh = idx_flat[:, None] == np.arange(E)[None, :]
    pos = (np.cumsum(oh, axis=0) * oh).sum(-1) - 1  # arrival position per expert
    keep_flat = pos < capacity  # (K*N,)

    # per-expert gather lists, in slot-major token order (kept only)
    tok_lists, w_lists = [], []
    flat_pos = np.zeros(K * N, dtype=np.int64)
    for e in range(E):
        sel = np.flatnonzero((idx_flat == e) & keep_flat)
        flat_pos[sel] = np.arange(sel.size)
        tok = sel % N
        slot = sel // N
        tok_lists.append(tok)
        w_lists.append(disp_w[tok, slot].astype(np.float32))

    # Switch-style aux loss
    assigned = oh.reshape(K, N, E).sum(0) > 0
    f = assigned.mean(0, dtype=np.float32)
    p_mean = probs.mean(0, dtype=np.float32)
    aux = np.float32(E * np.sum(f * p_mean, dtype=np.float32))

    return top_i, keep_flat, flat_pos, tok_lists, w_lists, aux


# ---------------------------------------------------------------------------
# Device kernel (built once per padded-size, shared SPMD across 8 cores)
# ---------------------------------------------------------------------------

def _build_nc(NP: int, D: int, DFF: int):
    import concourse.bass as bass  # noqa: F401
    import concourse.tile as tile
    from concourse import bacc, mybir
    from contextlib import ExitStack

    KT = D // 128     # k-tiles for the first matmul contraction
    FT = DFF // 128   # ff-tiles (contraction tiles for the second matmul)
    NT = NP // 128

    bf16 = mybir.dt.bfloat16
    f32 = mybir.dt.float32
    AF = mybir.ActivationFunctionType
    ALU = mybir.AluOpType

    nc = bacc.Bacc(
        "TRN2",
        target_bir_lowering=False,
        debug=False,
        enable_asserts=False,
        num_devices=E,
    )

    # xh is block-packed: per partition, each block's (kt-major) slab is one
    # contiguous 8KB run -> near-peak DMA efficiency, one DMA per block
    xh = nc.dram_tensor("xh", [128, KT * NP], bf16, kind="ExternalInput").ap()
    w1 = nc.dram_tensor("w1", [128, KT, DFF], bf16, kind="ExternalInput").ap()
    w2 = nc.dram_tensor("w2", [128, FT, D], bf16, kind="ExternalInput").ap()
    b1t = nc.dram_tensor("b1t", [128, FT], f32, kind="ExternalInput").ap()
    b2r = nc.dram_tensor("b2r", [128, D], f32, kind="ExternalInput").ap()
    wt = nc.dram_tensor("wt", [128, NT], f32, kind="ExternalInput").ap()
    y = nc.dram_tensor("y", [NP, D], f32, kind="ExternalOutput").ap()

    blocks = _block_plan(NP)

    with tile.TileContext(nc) as tc, ExitStack() as ctx:
        const = ctx.enter_context(tc.tile_pool(name="const", bufs=1))
        xpool = ctx.enter_context(tc.tile_pool(name="x", bufs=2))
        hpool = ctx.enter_context(tc.tile_pool(name="h", bufs=2))
        bpool = ctx.enter_context(tc.tile_pool(name="b2w", bufs=2))
        opool = ctx.enter_context(tc.tile_pool(name="o", bufs=3))
        # bufs is per tag; PSUM budget: ps1 3 + ps2 4 + warm-up 1 = 8 banks
        ps1 = ctx.enter_context(tc.tile_pool(name="ps1", bufs=3, space="PSUM"))
        ps2 = ctx.enter_context(tc.tile_pool(name="ps2", bufs=4, space="PSUM"))

        # PE warm-up: the first ~10us of execution are DMA-ramp-bound with the
        # PE idle, and the HAM clock gate then holds the first ~3.4us of real
        # matmuls at 1.2GHz.  Dummy matmuls on a memset scratch tile keep the
        # PE's activity window busy through the DMA wait so real matmuls start
        # at 2.4GHz.  80 N=128 matmuls span ~7us — ending just before the
        # measured ~17us first-data arrival, inside the ~3.4us re-throttle
        # window.
        wpool = ctx.enter_context(tc.tile_pool(name="warm", bufs=1))
        wpsp = ctx.enter_context(tc.tile_pool(name="warmps", bufs=1, space="PSUM"))
        warm_sb = wpool.tile([128, 512], bf16, name="warm_sb")
        nc.vector.memset(warm_sb, 0)
        warm_ps = wpsp.tile([128, 512], f32, name="warm_ps")
        for _ in range(48):
            nc.tensor.matmul(
                warm_ps[:, :128], lhsT=warm_sb[:, :128], rhs=warm_sb[:, :128],
                start=True, stop=True,
            )
        for _ in range(8):
            # longer dummies stretch coverage to the first data-dependent
            # matmul so the HAM clock gate never re-throttles
            nc.tensor.matmul(
                warm_ps, lhsT=warm_sb[:, :128], rhs=warm_sb,
                start=True, stop=True,
            )

        # Each dma_start costs ~0.6us of serialized issue time on its engine's
        # queue, so DMAs are kept few and spread across the three queues that
        # can issue them: W1/W2/y on gpsimd, x on sync, biases on scalar.
        #
        # W1 in (ff-group x kt-half) chunks, DMA'd ff-group-major so arrival
        # order matches phase-1 consumption order (psum ft needs only the
        # chunks of its own ff-group across all kt)
        FG = 4                  # ff-tiles per group
        NG = FT // FG           # number of ff groups
        KH = KT // 2            # kt per chunk
        w1_t = {}

        def load_w1_group(g):
            for kh in range(2):
                w1c = const.tile(
                    [128, KH * FG * 128], bf16, name=f"w1c{g}_{kh}", tag=f"w1c{g}_{kh}"
                )
                # single queue (gpsimd): issuing these from scalar delays the
                # gelus queued behind them on that engine (measured +12us)
                nc.gpsimd.dma_start(
                    w1c.rearrange("p (k f) -> p k f", k=KH),
                    w1[:, kh * KH : (kh + 1) * KH, g * FG * 128 : (g + 1) * FG * 128],
                )
                w1_t[(g, kh)] = w1c

        def w1_slice(ft, kt):
            g, fi = divmod(ft, FG)
            kh, ki = divmod(kt, KH)
            off = ki * FG * 128 + fi * 128
            return w1_t[(g, kh)][:, off : off + 128]

        # lazily-loaded small/late tensors (keep the critical W1+x DMA window
        # free of competing traffic)
        lazy: dict = {}

        def get_b1():
            if "b1" not in lazy:
                t = const.tile([128, FT], f32, name="b1_sb")
                nc.sync.dma_start(t, b1t)
                lazy["b1"] = t
            return lazy["b1"]

        def get_epi():
            if "b2" not in lazy:
                t = const.tile([128, D], f32, name="b2_sb")
                nc.sync.dma_start(t, b2r)
                lazy["b2"] = t
                t2 = const.tile([128, NT], f32, name="wt_sb")
                nc.sync.dma_start(t2, wt)
                lazy["wt"] = t2
            return lazy["b2"], lazy["wt"]

        FW = 4                  # ff-tiles per W2 chunk
        w2_t = []

        def load_w2():
            for c in range(FT // FW):
                w2c = const.tile([128, FW * D], bf16, name=f"w2c{c}", tag=f"w2c{c}")
                nc.gpsimd.dma_start(
                    w2c.rearrange("p (k f) -> p k f", k=FW),
                    w2[:, c * FW : (c + 1) * FW, :],
                )
                w2_t.append(w2c)

        def w2_slice(ft, lo, hi):
            c, fi = divmod(ft, FW)
            return w2_t[c][:, fi * D + lo : fi * D + hi]

        def emit_phase1(j0, tb, xoff, first=False):
            if first:
                # first block: x in kt-pair quarters interleaved with the W1
                # group loads, so the first matmuls only wait for ~500KB
                xq = []
                for q in range(KT // 2):
                    xc = xpool.tile([128, 2 * tb], bf16, name=f"xq{q}", tag=f"xq{q}")
                    nc.sync.dma_start(
                        xc, xh[:, xoff + q * 2 * tb : xoff + (q + 1) * 2 * tb]
                    )
                    if q < NG:
                        load_w1_group(q)
                    xq.append(xc)
                for g in range(KT // 2, NG):
                    load_w1_group(g)

                def rhs_slice(kt):
                    return xq[kt // 2][:, (kt % 2) * tb : (kt % 2 + 1) * tb]
            else:
                xt = xpool.tile([128, KT * tb], bf16, name="xt", tag="xt")
                nc.sync.dma_start(xt, xh[:, xoff : xoff + KT * tb])

                def rhs_slice(kt):
                    return xt[:, kt * tb : (kt + 1) * tb]

            b1_sb = get_b1()
            # h = gelu(x @ W1 + b1), laid out (ff, tok)
            h_tiles = []
            for ft in range(FT):
                ps = ps1.tile([128, tb], f32, name="ps", tag="ps1")
                for kt in range(KT):
                    nc.tensor.matmul(
                        ps,
                        lhsT=w1_slice(ft, kt),
                        rhs=rhs_slice(kt),
                        start=(kt == 0),
                        stop=(kt == KT - 1),
                    )
                ht = hpool.tile([128, tb], bf16, name="ht", tag=f"h{ft}")
                nc.scalar.activation(ht, ps, AF.Gelu, bias=b1_sb[:, ft : ft + 1])
                h_tiles.append(ht)
            return h_tiles

        def emit_phase2(j0, tb, h_tiles, last=False, penult=False):
            if not w2_t:
                load_w2()
            b2_sb, wt_sb = get_epi()
            # y = w * (h.T @ W2 + b2), laid out (tok, d); per-sub-group output
            # tiles + DMAs (on the mostly-idle gpsimd queue) so results drain
            # as soon as their two epilogue ops finish
            nsg = tb // 128
            for sg in range(nsg):
                g = j0 // 128 + sg
                wcol = wt_sb[:, g : g + 1]
                b2w = bpool.tile([128, D], f32, name="b2w", tag="b2w")
                nc.vector.tensor_scalar_mul(b2w, b2_sb, wcol)
                # final sub-group: separate half tiles so each half's output
                # DMA starts right after its own epilogue op, shortening the
                # end-of-kernel drain wait
                split = last and sg == nsg - 1
                if not split:
                    outt = opool.tile([128, D], f32, name="outt", tag="out")
                for half in range(D // 512):
                    p2 = ps2.tile([128, 512], f32, name="p2", tag="ps2")
                    for ft in range(FT):
                        nc.tensor.matmul(
                            p2,
                            lhsT=h_tiles[ft][:, sg * 128 : (sg + 1) * 128],
                            rhs=w2_slice(ft, half * 512, half * 512 + 512),
                            start=(ft == 0),
                            stop=(ft == FT - 1),
                        )
                    if split:
                        outh = opool.tile([128, 512], f32, name="outh", tag="outh")
                        nc.vector.scalar_tensor_tensor(
                            outh, p2, wcol, b2w[:, half * 512 : half * 512 + 512],
                            op0=ALU.mult, op1=ALU.add,
                        )
                        eng = nc.gpsimd if half == 0 else nc.sync
                        eng.dma_start(
                            y[j0 + sg * 128 : j0 + (sg + 1) * 128,
                              half * 512 : half * 512 + 512],
                            outh,
                        )
                    else:
                        nc.vector.scalar_tensor_tensor(
                            outt[:, half * 512 : half * 512 + 512],
                            p2,
                            wcol,
                            b2w[:, half * 512 : half * 512 + 512],
                            op0=ALU.mult,
                            op1=ALU.add,
                        )
                if not split:
                    # alternate output queues by sub-group parity so the
                    # end-of-kernel flush drains through both queues; the
                    # penultimate block's last sg goes via scalar (idle after
                    # the final gelu) for a 3-way tail flush
                    if penult and sg == nsg - 1:
                        eng = nc.scalar
                    else:
                        eng = nc.gpsimd if sg % 2 == 0 else nc.sync
                    eng.dma_start(
                        y[j0 + sg * 128 : j0 + (sg + 1) * 128, :], outt
                    )

        # software pipeline: phase 2 runs one block behind phase 1, so the
        # PE's early work only needs W1 + x while W2 is still streaming in
        pending = None
        xoff = 0
        for bi, (j0, tb) in enumerate(blocks):
            h_tiles = emit_phase1(j0, tb, xoff, first=(bi == 0))
            xoff += KT * tb
            if pending is not None:
                emit_phase2(*pending, penult=(bi == len(blocks) - 1))
            pending = (j0, tb, h_tiles)
        emit_phase2(*pending, last=True)

    nc.compile()
    return nc


# ---------------------------------------------------------------------------
# Entry point
# ---------------------------------------------------------------------------

_last_run = None  # BassKernelResults of the most recent device launch


def _ensure_axon_hooks():
    # bass_utils imports antenv.axon_hooks unconditionally when BASS_TRACE is
    # set under axon; provide a no-op registry if the image lacks the module
    try:
        import antenv.axon_hooks  # noqa: F401
    except Exception:
        import sys as _sys
        import types as _types

        m = _types.ModuleType("antenv.axon_hooks")
        m._h = None
        m.set_axon_ntff_profile_hook = lambda h: setattr(m, "_h", h)
        m.get_axon_ntff_profile_hook = lambda: getattr(m, "_h", None)
        _sys.modules["antenv.axon_hooks"] = m


def kernel(x, gate_w, W1, b1, W2, b2):
    _ensure_axon_hooks()
    from concourse.bass_utils import run_bass_kernel_spmd

    x = np.asarray(x, dtype=np.float32)
    gate_w = np.asarray(gate_w, dtype=np.float32)
    W1 = np.asarray(W1, dtype=np.float32)
    b1 = np.asarray(b1, dtype=np.float32)
    W2 = np.asarray(W2, dtype=np.float32)
    b2 = np.asarray(b2, dtype=np.float32)

    B, T, D = x.shape
    DFF = W1.shape[2]
    N = B * T
    x_flat = x.reshape(N, D)

    top_i, keep_flat, flat_pos, tok_lists, w_lists, aux = _route(x_flat, gate_w)

    NP = max(128, max(-(-len(t) // 128) * 128 for t in tok_lists))
    KT, FT, NT = D // 128, DFF // 128, NP // 128

    key = (NP, D, DFF)
    if key not in _NC_CACHE:
        _NC_CACHE[key] = _build_nc(NP, D, DFF)
    nc = _NC_CACHE[key]

    bf16 = ml_dtypes.bfloat16
    x_bf = x_flat.astype(bf16)
    blocks = _block_plan(NP)

    in_maps = []
    for e in range(E):
        tok = tok_lists[e]
        n = len(tok)
        xg = np.zeros((NP, D), dtype=bf16)
        xg[:n] = x_bf[tok]
        # (NP, D) -> (128, KT, NP): xT[p, kt, j] = xg[j, kt*128 + p]
        xT_h = xg.reshape(NP, KT, 128).transpose(2, 1, 0)
        # block-packed: per partition, block (j0, tb) occupies one contiguous
        # kt-major slab [kt*tb + j]
        xh_h = np.empty((128, KT * NP), dtype=bf16)
        off = 0
        for j0, tb in blocks:
            xh_h[:, off : off + KT * tb] = xT_h[:, :, j0 : j0 + tb].reshape(128, KT * tb)
            off += KT * tb
        w1_h = np.ascontiguousarray(
            W1[e].reshape(KT, 128, DFF).transpose(1, 0, 2).astype(bf16)
        )
        w2_h = np.ascontiguousarray(
            W2[e].reshape(FT, 128, D).transpose(1, 0, 2).astype(bf16)
        )
        b1_h = np.ascontiguousarray(b1[e].reshape(FT, 128).T)
        b2_h = np.ascontiguousarray(np.broadcast_to(b2[e], (128, D)))
        wt_h = np.zeros((NT, 128), dtype=np.float32)
        wt_h.reshape(-1)[:n] = w_lists[e]
        wt_h = np.ascontiguousarray(wt_h.T)
        in_maps.append(
            {"xh": xh_h, "w1": w1_h, "w2": w2_h, "b1t": b1_h, "b2r": b2_h, "wt": wt_h}
        )

    res = run_bass_kernel_spmd(nc, in_maps, core_ids=list(range(E)))
    global _last_run
    _last_run = res

    y_all = np.stack([r["y"] for r in res.results]).reshape(E * NP, D)

    out = np.zeros((N, D), dtype=np.float32)
    ar = np.arange(N)
    for k in range(K):
        kk = keep_flat[k * N : (k + 1) * N]
        src = top_i[:, k].astype(np.int64) * NP + flat_pos[k * N + ar]
        contrib = y_all[src]
        contrib[~kk] = 0.0
        out += contrib

    return out.reshape(B, T, D), aux
